# revision 1
# baseline (speedup 1.0000x reference)
"""Trainium2 Bass kernel for nn_AttentionBlock (B=16, C=512, H=W=64, 8 heads).

Channel-attention block: GroupNorm(8 groups) -> 1x1 qkv -> scores over
channel dims (contract spatial N=4096) -> softmax -> att @ v -> 1x1 out
projection -> residual.

Sharding: data-parallel over batch. 16 batches / 8 cores = 2 per core.
No collectives. Each core runs the identical program on its 2 batches.

Layouts on device (per batch):
  x     [C, N] fp32, 4 channel-chunk tiles of [128, 4096]
  h     (groupnorm output) same layout, bf16
  q,k   [N, 2C] orientation (spatial on partitions), bf16, transient tiles
  v     [C, N] bf16, resident
  scores 8 heads of [64, 64] packed into two [128, 128] psum tiles
  hv    [C, N] bf16 via paired-head matmuls
  out   = w_out @ hv + (w_out @ (att @ b_v) + b_out) + x   (residual)

All matmuls bf16 inputs with fp32 psum accumulation; groupnorm stats,
softmax, and the residual path are fp32.
"""

import numpy as np
import ml_dtypes

import concourse.bacc as bacc
import concourse.tile as tile
from concourse import mybir
from concourse.bass_utils import run_bass_kernel_spmd
from concourse.masks import make_identity

BF = mybir.dt.bfloat16
F32 = mybir.dt.float32
AX = mybir.AxisListType
OP = mybir.AluOpType
AF = mybir.ActivationFunctionType

C = 512
NH = 8
D = 64  # head dim
G = 8   # groupnorm groups
CK = C // 128  # 4 channel chunks
EPS = 1e-5
N_CORES = 8

# attT slot coords inside a [128,128] attT tile, per chunk parity.
# chunk ck holds heads (2ck, 2ck+1); tile tt = ck // 2.
# even head lhsT lives at partitions 0:64, odd head at partitions 64:128.
_EVEN_SLOT = {0: (0, 0), 1: (0, 64)}   # ck%2 -> (prow, colstart)
_ODD_SLOT = {0: (64, 64), 1: (64, 0)}
# scores placement: local head l (0..3) -> (prow, colstart) in scores tile
_SCORE_SLOT = {0: (0, 0), 1: (64, 64), 2: (64, 0), 3: (0, 64)}


def build_program(B=2, N=4096, debug=False):
    SP = N // 128   # spatial chunks for qk/scores
    NT = N // 512   # 512-col tiles
    SUB = N // 512  # bn_stats subgroups (free dim <= 512)
    scale = float(1.0 / np.sqrt(D))

    nc = bacc.Bacc("TRN2", target_bir_lowering=False, debug=debug,
                   num_devices=N_CORES)

    x_d = nc.dram_tensor("x", [B, C, N], F32, kind="ExternalInput")
    wqk_d = nc.dram_tensor("wqkT", [C, 2 * C], BF, kind="ExternalInput")
    wv_d = nc.dram_tensor("wvT", [C, C], BF, kind="ExternalInput")
    wo_d = nc.dram_tensor("woT", [C, C], BF, kind="ExternalInput")
    bqk_d = nc.dram_tensor("bqk", [1, 2 * C], BF, kind="ExternalInput")
    bv_d = nc.dram_tensor("bv", [C, 1], BF, kind="ExternalInput")
    bo_d = nc.dram_tensor("bo", [C, 1], F32, kind="ExternalInput")
    gam_d = nc.dram_tensor("gamma", [C, 1], F32, kind="ExternalInput")
    bet_d = nc.dram_tensor("beta", [C, 1], F32, kind="ExternalInput")
    indf_d = nc.dram_tensor("indf", [C, G], F32, kind="ExternalInput")
    indb_d = nc.dram_tensor("indb", [G, C], F32, kind="ExternalInput")
    out_d = nc.dram_tensor("out", [B, C, N], F32, kind="ExternalOutput")

    with tile.TileContext(nc) as tc:
        import contextlib
        ctx = contextlib.ExitStack()
        with ctx:
            persist = ctx.enter_context(tc.tile_pool(name="persist", bufs=1))
            big = ctx.enter_context(tc.tile_pool(name="big", bufs=1))
            mid = ctx.enter_context(tc.tile_pool(name="mid", bufs=3))
            small = ctx.enter_context(tc.tile_pool(name="small", bufs=1))
            ps_qk = ctx.enter_context(
                tc.tile_pool(name="ps_qk", bufs=3, space="PSUM"))
            ps_sc = ctx.enter_context(
                tc.tile_pool(name="ps_sc", bufs=1, space="PSUM"))
            ps_big = ctx.enter_context(
                tc.tile_pool(name="ps_big", bufs=2, space="PSUM"))

            # ---- persistent: weights / constants ----
            wqk = []
            wv = []
            wo = []
            bv_sb = []
            bo_sb = []
            gam = []
            bet = []
            for k in range(CK):
                t = persist.tile([128, 2 * C], BF, tag=f"wqk{k}")
                nc.gpsimd.dma_start(out=t, in_=wqk_d.ap()[k * 128:(k + 1) * 128, :])
                wqk.append(t)
                t = persist.tile([128, C], BF, tag=f"wv{k}")
                nc.gpsimd.dma_start(out=t, in_=wv_d.ap()[k * 128:(k + 1) * 128, :])
                wv.append(t)
                t = persist.tile([128, C], BF, tag=f"wo{k}")
                nc.gpsimd.dma_start(out=t, in_=wo_d.ap()[k * 128:(k + 1) * 128, :])
                wo.append(t)
                t = persist.tile([128, 1], BF, tag=f"bv{k}")
                nc.gpsimd.dma_start(out=t, in_=bv_d.ap()[k * 128:(k + 1) * 128, :])
                bv_sb.append(t)
                t = persist.tile([128, 1], F32, tag=f"bo{k}")
                nc.gpsimd.dma_start(out=t, in_=bo_d.ap()[k * 128:(k + 1) * 128, :])
                bo_sb.append(t)
                t = persist.tile([128, 1], F32, tag=f"gam{k}")
                nc.gpsimd.dma_start(out=t, in_=gam_d.ap()[k * 128:(k + 1) * 128, :])
                gam.append(t)
                t = persist.tile([128, 1], F32, tag=f"bet{k}")
                nc.gpsimd.dma_start(out=t, in_=bet_d.ap()[k * 128:(k + 1) * 128, :])
                bet.append(t)
            # q/k bias replicated across all 128 partitions (spatial rows)
            import concourse.bass as bass
            bqk_rep = persist.tile([128, 2 * C], BF, tag="bqk_rep")
            _bqk_ap = bqk_d.ap()
            nc.gpsimd.dma_start(
                out=bqk_rep,
                in_=bass.AP(tensor=_bqk_ap.tensor, offset=_bqk_ap.offset,
                            ap=[[0, 128], [1, 2 * C]]))

            zero1 = persist.tile([1, 128], BF, tag="zero1")
            nc.gpsimd.memset(zero1, 0.0)
            zrhs256 = persist.tile([1, 256], BF, tag="zrhs256")
            nc.gpsimd.memset(zrhs256, 0.0)
            ident = persist.tile([128, 128], BF, tag="ident")
            make_identity(nc, ident)
            eps_t = persist.tile([128, 1], F32, tag="eps")
            nc.gpsimd.memset(eps_t, EPS)
            # group indicator matrices (groupnorm cross-partition reduce)
            indf = []
            for k in range(CK):
                t = persist.tile([128, G], F32, tag=f"indf{k}")
                nc.gpsimd.dma_start(
                    out=t, in_=indf_d.ap()[k * 128:(k + 1) * 128, :])
                indf.append(t)
            indb = persist.tile([G, C], F32, tag="indb")
            nc.gpsimd.dma_start(out=indb, in_=indb_d.ap())

            # ---- per-batch phases (emitted software-pipelined below) ----
            def phase_norm(b):
                # x load (split DMAs so bn_stats can start on early columns)
                xs = []
                for k in range(CK):
                    t = big.tile([128, N], F32, tag=f"x{k}")
                    xq = min(1024, N)
                    for q4 in range(0, N, xq):
                        nc.sync.dma_start(
                            out=t[:, q4:q4 + xq],
                            in_=x_d.ap()[b, k * 128:(k + 1) * 128,
                                         q4:q4 + xq])
                    xs.append(t)

                # groupnorm stats: per-partition mean/var via bn_stats
                mvs = []
                for k in range(CK):
                    st = small.tile([128, SUB, 6], F32, tag=f"st{k}")
                    for j in range(SUB):
                        nc.vector.bn_stats(
                            out=st[:, j, :], in_=xs[k][:, j * 512:(j + 1) * 512])
                    mv = small.tile([128, 2], F32, tag=f"mv{k}")
                    nc.vector.bn_aggr(out=mv, in_=st)
                    mvs.append(mv)
                # rhs2: col0 = mean_p, col1 = mean_p^2 + var_p = E[x^2]_p
                rhs2s = []
                for k in range(CK):
                    r2 = small.tile([128, 2], F32, tag=f"r2{k}")
                    nc.gpsimd.tensor_copy(out=r2[:, 0:1], in_=mvs[k][:, 0:1])
                    nc.vector.scalar_tensor_tensor(
                        out=r2[:, 1:2], in0=mvs[k][:, 0:1],
                        scalar=mvs[k][:, 0:1], in1=mvs[k][:, 1:2],
                        op0=OP.mult, op1=OP.add)
                    rhs2s.append(r2)
                # cross-partition reduce to per-group stats [8, 2]
                pg = ps_big.tile([G, 2], F32, tag="pbig")
                for k in range(CK):
                    nc.tensor.matmul(pg, indf[k], rhs2s[k],
                                     start=(k == 0), stop=(k == CK - 1))
                sg = small.tile([G, 2], F32, tag="sg")
                nc.vector.tensor_copy(out=sg, in_=pg)
                t2 = small.tile([G, 1], F32, tag="t2")
                nc.vector.tensor_mul(out=t2, in0=sg[:, 0:1], in1=sg[:, 0:1])
                vs = small.tile([G, 1], F32, tag="vs")
                nc.vector.tensor_sub(out=vs, in0=sg[:, 1:2], in1=t2)
                # rstd = exp(-0.5 * ln(var + eps)); Ln/Exp share a table set
                lnv = small.tile([G, 1], F32, tag="lnv")
                nc.scalar.activation(out=lnv, in_=vs, func=AF.Ln,
                                     bias=eps_t[0:G, :], scale=1.0)
                rstd = small.tile([G, 1], F32, tag="rstd")
                nc.scalar.activation(out=rstd, in_=lnv, func=AF.Exp, scale=-0.5)
                bcr = small.tile([G, 2], F32, tag="bcr")
                nc.gpsimd.tensor_copy(out=bcr[:, 0:1], in_=sg[:, 0:1])
                nc.gpsimd.tensor_copy(out=bcr[:, 1:2], in_=rstd)
                # broadcast group stats back to channels; affine coeffs
                scs = []
                nbs = []
                for k in range(CK):
                    pbc = ps_big.tile([128, 2], F32, tag="pbig")
                    nc.tensor.matmul(pbc, indb[:, k * 128:(k + 1) * 128], bcr,
                                     start=True, stop=True)
                    sc = small.tile([128, 1], F32, tag=f"sc{k}")
                    nc.vector.tensor_mul(out=sc, in0=pbc[:, 1:2], in1=gam[k])
                    t4 = small.tile([128, 1], F32, tag=f"t4{k}")
                    nc.vector.tensor_scalar_mul(out=t4, in0=pbc[:, 0:1],
                                                scalar1=sc)
                    nb = small.tile([128, 1], F32, tag=f"nb{k}")
                    nc.vector.tensor_sub(out=nb, in0=bet[k], in1=t4)
                    scs.append(sc)
                    nbs.append(nb)

                # normalize: h = x * scale_c + bias_c  (bf16).
                # Column-major loop order: the first qk matmul needs the
                # first 128 columns of ALL FOUR chunks, so producing columns
                # across chunks first lets the consumer start ~9us earlier
                # than chunk-major order would.
                hs = []
                for k in range(CK):
                    hs.append(big.tile([128, N], BF, tag=f"h{k}",
                                       name=f"h{k}"))
                for t in range(NT):
                    sl = slice(t * 512, (t + 1) * 512)
                    for k in range(CK):
                        nc.vector.tensor_scalar(
                            out=hs[k][:, sl], in0=xs[k][:, sl],
                            scalar1=scs[k], scalar2=nbs[k],
                            op0=OP.mult, op1=OP.add)
                return hs

            def phase_qkv_setup(b):
                # scores accumulators: both packed tiles share one psum bank
                Tsc = ps_sc.tile([128, 256], F32, tag="sc01")
                T0 = Tsc[:, 0:128]
                T1 = Tsc[:, 128:256]
                # one full-width zeroing matmul: marks the bank's pending-zero
                # bits and writes 0 everywhere; every scores matmul overlaps
                # its AP, so ordering is guaranteed, and all quadrant matmuls
                # can then accumulate in any order.
                nc.tensor.matmul(Tsc, zero1, zrhs256, start=True, stop=False,
                                 skip_group_check=True)
                vsb = []
                for k in range(CK):
                    vsb.append(big.tile([128, N], BF, tag=f"v{k}",
                                        name=f"v{k}"))
                return T0, T1, vsb

            def qk_chunk(b, hs, s):
                # qk projection for one 128-row spatial chunk
                qk = mid.tile([128, 2 * C], BF, tag="qk", bufs=4)
                for half in range(2):
                    # one-bank psum tiles (3 rotating slots) so the next
                    # chunk's matmuls never wait on this chunk's evac
                    pqk = ps_qk.tile([128, 512], F32, tag="pqk")
                    wseg = slice(half * 512, (half + 1) * 512)
                    for k in range(CK):
                        nc.tensor.matmul(
                            pqk, hs[k][:, s * 128:(s + 1) * 128],
                            wqk[k][:, wseg], start=(k == 0),
                            stop=(k == CK - 1))
                    nc.scalar.copy(out=qk[:, wseg], in_=pqk)
                # q/k bias add (bf16 tensor_tensor runs in DVE 2x mode)
                nc.vector.tensor_add(out=qk, in0=qk, in1=bqk_rep)
                return qk

            def emit_scores(qk, T0, T1):
                for h in range(NH):
                    tt, l = divmod(h, 4)
                    T = T0 if tt == 0 else T1
                    pr, cs = _SCORE_SLOT[l]
                    nc.tensor.matmul(
                        T[pr:pr + 64, cs:cs + 64],
                        qk[:, h * 64:(h + 1) * 64],
                        qk[:, 512 + h * 64:512 + (h + 1) * 64],
                        start=False, stop=False, skip_group_check=True,
                        tile_position=(0, pr))

            def phase_qkv_run(b, hs, T0, T1, vsb, s0, s1):
                # qk + scores, with the v projection interleaved (one 512-col
                # block per 4 spatial chunks) so h slices are fully consumed
                # — and released for the next batch's normalize — as the
                # loop advances.
                for s in range(s0, s1):
                    qk = qk_chunk(b, hs, s)
                    emit_scores(qk, T0, T1)
                    if s % 4 == 3:
                        t = s // 4
                        hsl = slice(t * 512, (t + 1) * 512)
                        for oc in range(CK):
                            pv = ps_big.tile([128, 512], F32, tag="pbig")
                            for k in range(CK):
                                nc.tensor.matmul(
                                    pv, wv[k][:, oc * 128:(oc + 1) * 128],
                                    hs[k][:, hsl], start=(k == 0),
                                    stop=(k == CK - 1))
                            # tensor_scalar has a 2x-mode uop (CAST is 1x)
                            nc.vector.tensor_scalar_mul(
                                out=vsb[oc][:, hsl], in0=pv, scalar1=1.0)

            def phase_att_out(b, T0, T1, vsb):
                # softmax + transpose -> attT (bf16)
                # softmax without max-subtraction: logits = S/8 are bounded
                # well inside fp32 exp range for this distribution.
                attTs = []
                for tt, T in enumerate([T0, T1]):
                    p_f = small.tile([128, 128], F32, tag=f"p{tt}")
                    att_bf = small.tile([128, 128], BF, tag=f"abf{tt}")
                    nc.scalar.activation(out=p_f, in_=T, func=AF.Exp,
                                         scale=scale)
                    rsum = small.tile([128, 2], F32, tag=f"rsum{tt}")
                    nc.vector.reduce_sum(
                        out=rsum,
                        in_=p_f.rearrange("p (h e) -> p h e", h=2),
                        axis=AX.X)
                    rinv = small.tile([128, 2], F32, tag=f"rinv{tt}")
                    nc.vector.reciprocal(out=rinv, in_=rsum)
                    for half in range(2):
                        sl = slice(half * 64, (half + 1) * 64)
                        nc.vector.tensor_scalar_mul(
                            out=att_bf[:, sl], in0=p_f[:, sl],
                            scalar1=rinv[:, half:half + 1])
                    ptr = ps_big.tile([128, 128], BF, tag="pbig")
                    nc.tensor.transpose(ptr, att_bf, ident)
                    aT = small.tile([128, 128], BF, tag=f"aT{tt}")
                    nc.vector.tensor_copy(out=aT, in_=ptr)
                    attTs.append(aT)

                # c = att @ b_v per head -> [C, 1] fp32; folded into the hv
                # evacuation as a per-partition bias (hv' = hv + c), which
                # makes w_out @ hv' carry the whole v-bias term so the output
                # only needs + b_out + x afterwards.
                csb = []
                for k in range(CK):
                    pcv = ps_big.tile([128, 1], F32, tag="pbig")
                    aT = attTs[k // 2]
                    epr, ecs = _EVEN_SLOT[k % 2]
                    opr, ocs = _ODD_SLOT[k % 2]
                    nc.tensor.matmul(
                        pcv[0:64, :], aT[epr:epr + 64, ecs:ecs + 64],
                        bv_sb[k][0:64, :], start=True, stop=True,
                        tile_position=(0, 0), skip_group_check=True)
                    nc.tensor.matmul(
                        pcv[64:128, :], aT[opr:opr + 64, ocs:ocs + 64],
                        bv_sb[k][64:128, :], start=True, stop=True,
                        tile_position=(64, 64), skip_group_check=True)
                    ct = small.tile([128, 1], F32, tag=f"c{k}")
                    nc.vector.tensor_copy(out=ct, in_=pcv)
                    csb.append(ct)

                # hv = att @ v, out = w_out @ hv + btot + x
                for t in range(NT):
                    hsl = slice(t * 512, (t + 1) * 512)
                    hvs = []
                    for k in range(CK):
                        phv = ps_big.tile([128, 512], F32, tag="pbig")
                        aT = attTs[k // 2]
                        epr, ecs = _EVEN_SLOT[k % 2]
                        opr, ocs = _ODD_SLOT[k % 2]
                        nc.tensor.matmul(
                            phv[0:64, :], aT[epr:epr + 64, ecs:ecs + 64],
                            vsb[k][0:64, hsl], start=True, stop=True,
                            tile_position=(0, 0), skip_group_check=True)
                        nc.tensor.matmul(
                            phv[64:128, :], aT[opr:opr + 64, ocs:ocs + 64],
                            vsb[k][64:128, hsl], start=True, stop=True,
                            tile_position=(64, 64), skip_group_check=True)
                        hv = mid.tile([128, 512], BF, tag=f"hv{k}", bufs=2)
                        # evacuate + add the folded v-bias (DVE 2x mode)
                        nc.vector.tensor_scalar_add(out=hv, in0=phv,
                                                    scalar1=csb[k])
                        hvs.append(hv)
                    for oc in range(CK):
                        # out-psum gets its own 2-slot tag so it never waits
                        # on hv-psum recycling (and vice versa)
                        po = ps_big.tile([128, 512], F32, tag="pout")
                        for k in range(CK):
                            nc.tensor.matmul(
                                po, wo[k][:, oc * 128:(oc + 1) * 128], hvs[k],
                                start=(k == 0), stop=(k == CK - 1))
                        xr = mid.tile([128, 512], F32, tag="xr")
                        nc.sync.dma_start(
                            out=xr,
                            in_=x_d.ap()[b, oc * 128:(oc + 1) * 128, hsl])
                        fin = mid.tile([128, 512], F32, tag="fin")
                        nc.vector.scalar_tensor_tensor(
                            out=fin, in0=po, scalar=bo_sb[oc], in1=xr,
                            op0=OP.add, op1=OP.add)
                        # non-final batches store via the idle gpsimd queue so
                        # they never delay the next batch's x loads on the
                        # sync queue; the last batch stores via sync (HWDGE)
                        # to shorten the kernel-tail drain
                        dma_eng = nc.gpsimd if b < B - 1 else nc.sync
                        dma_eng.dma_start(
                            out=out_d.ap()[b, oc * 128:(oc + 1) * 128, hsl],
                            in_=fin)

            # software-pipelined emission: batch b+1's stats/normalize AND
            # its first PRE qk-projection chunks (scores deferred to avoid
            # an in-order queue cycle) are emitted ahead of batch b's
            # softmax/hv/out, so the tensor engine has filler work while
            # batch b's softmax chain runs on DVE/ACT.
            PRE = min(3, SP)
            hs_b = phase_norm(0)
            st_b = phase_qkv_setup(0)
            phase_qkv_run(0, hs_b, *st_b, 0, SP)
            for b in range(1, B):
                hs_n = phase_norm(b)
                stash = [qk_chunk(b, hs_n, s) for s in range(PRE)]
                phase_att_out(b - 1, *st_b)
                st_b = phase_qkv_setup(b)
                for qk in stash:
                    emit_scores(qk, st_b[0], st_b[1])
                phase_qkv_run(b, hs_n, *st_b, PRE, SP)
                hs_b = hs_n
            phase_att_out(B - 1, *st_b)

    nc.compile()
    return nc


def make_indicators():
    """Host-built groupnorm reduce/broadcast indicator matrices."""
    ch = np.arange(C)
    grp = ch // (C // G)
    indf = np.zeros((C, G), np.float32)
    indf[ch, grp] = 1.0 / (C // G)
    indb = np.zeros((G, C), np.float32)
    indb[grp, ch] = 1.0
    return indf, indb


_PROGRAM = None


def _get_program():
    global _PROGRAM
    if _PROGRAM is None:
        _PROGRAM = build_program()
    return _PROGRAM


def kernel(x, gamma, beta, w_qkv, b_qkv, w_out, b_out):
    x = np.asarray(x)
    B, C_, H, W = x.shape
    N = H * W
    assert C_ == C and B == 16 and N == 4096
    nc = _get_program()

    bf = ml_dtypes.bfloat16
    w_qkv = np.asarray(w_qkv, dtype=np.float32)
    wqkT = np.ascontiguousarray(w_qkv[:2 * C].T).astype(bf)
    wvT = np.ascontiguousarray(w_qkv[2 * C:].T).astype(bf)
    woT = np.ascontiguousarray(np.asarray(w_out, dtype=np.float32).T).astype(bf)
    b_qkv = np.asarray(b_qkv, dtype=np.float32)
    bqk = np.ascontiguousarray(b_qkv[:2 * C].reshape(1, -1)).astype(bf)
    bv = np.ascontiguousarray(b_qkv[2 * C:].reshape(-1, 1)).astype(bf)
    bo = np.ascontiguousarray(np.asarray(b_out, np.float32).reshape(-1, 1))
    gam = np.ascontiguousarray(np.asarray(gamma, np.float32).reshape(-1, 1))
    bet = np.ascontiguousarray(np.asarray(beta, np.float32).reshape(-1, 1))
    xr = np.ascontiguousarray(x.reshape(B, C, N).astype(np.float32))

    indf, indb = make_indicators()
    bpc = B // N_CORES
    in_maps = []
    for c in range(N_CORES):
        in_maps.append({
            "x": xr[c * bpc:(c + 1) * bpc],
            "wqkT": wqkT, "wvT": wvT, "woT": woT,
            "bqk": bqk, "bv": bv, "bo": bo,
            "gamma": gam, "beta": bet,
            "indf": indf, "indb": indb,
        })
    res = run_bass_kernel_spmd(nc, in_maps, core_ids=list(range(N_CORES)))
    out = np.concatenate([res.results[c]["out"] for c in range(N_CORES)],
                         axis=0)
    return out.reshape(B, C_, H, W).astype(np.float32)



# revision 7
# speedup vs baseline: 1.1376x; 1.1376x over previous
"""Trainium2 Bass kernel for nn_AttentionBlock (B=16, C=512, H=W=64, 8 heads).

Channel-attention block: GroupNorm(8 groups) -> 1x1 qkv -> scores over
channel dims (contract spatial N=4096) -> softmax -> att @ v -> 1x1 out
projection -> residual.

Sharding: data-parallel over batch, 2 per core, no collectives.

Key structure (vs a direct port of the reference):
  * scores are computed via the Gram matrix G = h @ h^T:
        S = Wq G Wk^T + qs x bk + bq x (ks + N*bk),  qs/ks = Wq/Wk @ rowsum(h)
    which costs C*C*N MACs (half of the q,k projections) and removes the
    [N, 2C] qk psum evacuation entirely.  G is built from PE transposes of
    h (spatial-on-partition tiles) accumulated over 32 spatial chunks.
  * v and out projections run in fp8 (e4m3) DoubleRow perf mode: weights
    are scaled by 32 host-side (avoids subnormals), inputs h/hv are stored
    fp8 in a paired-chunk layout [128, 2, N], and each matmul contracts
    256 channels at 2x throughput.  The 1/32 unscale folds into the psum
    evacuations.  (q,k stay bf16: softmax amplifies fp8 noise there.)
  * att is kept block-diagonal per channel-chunk ([128,128] tiles with two
    64x64 head blocks on the diagonal), so att@v and att@b_v are single
    full-width 128-contraction matmuls per (chunk, t-block).
  * x is loaded in bf16 (halves the startup DMA); the residual re-loads
    x in fp32 per 512-col block.
  * engine split: PE matmuls; ACT normalize+v/out/G/T evacs+exp; DVE
    bn_stats+hT/hv evacs+softmax small ops; Pool h8 normalize, residual
    adds and output stores.
"""

import numpy as np
import ml_dtypes

import concourse.bacc as bacc
import concourse.tile as tile
from concourse import mybir
from concourse.bass_utils import run_bass_kernel_spmd
from concourse.masks import make_identity

BF = mybir.dt.bfloat16
F8 = mybir.dt.float8e4
F32 = mybir.dt.float32
AX = mybir.AxisListType
OP = mybir.AluOpType
AF = mybir.ActivationFunctionType
DR = mybir.MatmulPerfMode.DoubleRow

C = 512
NH = 8
D = 64
G = 8
CK = C // 128   # 4 channel chunks
NP = 2          # chunk pairs for fp8 DoubleRow
EPS = 1e-5
N_CORES = 8
WSC = 32.0      # fp8 weight scale
IWSC = float(1.0 / WSC)


def build_program(B=2, N=4096, debug=False):
    SP = N // 128   # 32 spatial chunks
    NT = N // 512   # 8 t-blocks
    SUB = N // 512
    scale = float(1.0 / np.sqrt(D))

    nc = bacc.Bacc("TRN2", target_bir_lowering=False, debug=debug,
                   num_devices=N_CORES)

    xbf_d = nc.dram_tensor("xbf", [B, C, N], BF, kind="ExternalInput")
    xf_d = nc.dram_tensor("xf", [B, C, N], F32, kind="ExternalInput")
    wqkT_d = nc.dram_tensor("wqkT", [C, 2 * C], BF, kind="ExternalInput")
    wv8_d = nc.dram_tensor("wv8", [NP, 128, 2, C], F8, kind="ExternalInput")
    wo8_d = nc.dram_tensor("wo8", [NP, 128, 2, C], F8, kind="ExternalInput")
    bqk_d = nc.dram_tensor("bqkr", [1, 2 * C], BF, kind="ExternalInput")
    bv_d = nc.dram_tensor("bv", [C, 1], BF, kind="ExternalInput")
    bo_d = nc.dram_tensor("bo", [C, 1], F32, kind="ExternalInput")
    gam_d = nc.dram_tensor("gamma", [C, 1], F32, kind="ExternalInput")
    bet_d = nc.dram_tensor("beta", [C, 1], F32, kind="ExternalInput")
    indf_d = nc.dram_tensor("indf", [C, G], F32, kind="ExternalInput")
    indb_d = nc.dram_tensor("indb", [G, C], F32, kind="ExternalInput")
    out_d = nc.dram_tensor("out", [B, C, N], F32, kind="ExternalOutput")

    with tile.TileContext(nc) as tc:
        import contextlib
        ctx = contextlib.ExitStack()
        with ctx:
            persist = ctx.enter_context(tc.tile_pool(name="persist", bufs=1))
            xpool = ctx.enter_context(tc.tile_pool(name="xpool", bufs=1))
            hpool = ctx.enter_context(tc.tile_pool(name="hpool", bufs=1))
            vpool = ctx.enter_context(tc.tile_pool(name="vpool", bufs=2))
            gpool = ctx.enter_context(tc.tile_pool(name="gpool", bufs=1))
            mid = ctx.enter_context(tc.tile_pool(name="mid", bufs=3))
            small = ctx.enter_context(tc.tile_pool(name="small", bufs=1))
            # PSUM: G 4 banks + tr 1 + vp 1 + av 1 + op 1 + sc .5 = 8 banks
            ps_g = ctx.enter_context(
                tc.tile_pool(name="ps_g", bufs=1, space="PSUM"))
            ps_tr = ctx.enter_context(
                tc.tile_pool(name="ps_tr", bufs=1, space="PSUM"))
            ps_w = ctx.enter_context(
                tc.tile_pool(name="ps_w", bufs=1, space="PSUM"))

            # ---- persistent weights / constants ----
            wqk = []
            for k in range(CK):
                t = persist.tile([128, 2 * C], BF, tag=f"wqk{k}")
                nc.gpsimd.dma_start(out=t, in_=wqkT_d.ap()[k * 128:(k + 1) * 128, :])
                wqk.append(t)
            wv8 = []
            wo8 = []
            for p in range(NP):
                t = persist.tile([128, 2, C], F8, tag=f"wv8{p}")
                nc.gpsimd.dma_start(out=t, in_=wv8_d.ap()[p])
                wv8.append(t)
                t = persist.tile([128, 2, C], F8, tag=f"wo8{p}")
                nc.gpsimd.dma_start(out=t, in_=wo8_d.ap()[p])
                wo8.append(t)
            bqkr = persist.tile([1, 2 * C], BF, tag="bqkr")
            nc.gpsimd.dma_start(out=bqkr, in_=bqk_d.ap())
            bv_sb = []
            bo_sb = []
            gam = []
            bet = []
            indf = []
            for k in range(CK):
                t = persist.tile([128, 1], BF, tag=f"bv{k}")
                nc.gpsimd.dma_start(out=t, in_=bv_d.ap()[k * 128:(k + 1) * 128, :])
                bv_sb.append(t)
                t = persist.tile([128, 1], F32, tag=f"bo{k}")
                nc.gpsimd.dma_start(out=t, in_=bo_d.ap()[k * 128:(k + 1) * 128, :])
                bo_sb.append(t)
                t = persist.tile([128, 1], F32, tag=f"gam{k}")
                nc.gpsimd.dma_start(out=t, in_=gam_d.ap()[k * 128:(k + 1) * 128, :])
                gam.append(t)
                t = persist.tile([128, 1], F32, tag=f"bet{k}")
                nc.gpsimd.dma_start(out=t, in_=bet_d.ap()[k * 128:(k + 1) * 128, :])
                bet.append(t)
                t = persist.tile([128, G], F32, tag=f"indf{k}")
                nc.gpsimd.dma_start(out=t, in_=indf_d.ap()[k * 128:(k + 1) * 128, :])
                indf.append(t)
            indb = persist.tile([G, C], F32, tag="indb")
            nc.gpsimd.dma_start(out=indb, in_=indb_d.ap())
            ident = persist.tile([128, 128], BF, tag="ident")
            make_identity(nc, ident)
            eps_t = persist.tile([128, 1], F32, tag="eps")
            nc.gpsimd.memset(eps_t, EPS)
            # block-diag att tiles: off-diagonal quadrants stay zero forever
            att_bf = []
            for k in range(CK):
                t = persist.tile([128, 128], BF, tag=f"attb{k}")
                nc.gpsimd.memset(t, 0.0)
                att_bf.append(t)

            # ---------------- phase helpers ----------------
            def load_x(b):
                xs = []
                for k in range(CK):
                    t = xpool.tile([128, N], BF, tag=f"x{k}")
                    for q2 in range(0, N, 2048):
                        nc.sync.dma_start(
                            out=t[:, q2:q2 + 2048],
                            in_=xbf_d.ap()[b, k * 128:(k + 1) * 128,
                                           q2:q2 + 2048])
                    xs.append(t)
                return xs

            def emit_bn_stats(xs, k, j):
                # one bn_stats op on a 512-col block; st tile per chunk
                if j == 0:
                    st = small.tile([128, SUB, 6], F32, tag=f"st{k}")
                    emit_bn_stats.st[k] = st
                st = emit_bn_stats.st[k]
                nc.vector.bn_stats(out=st[:, j, :],
                                   in_=xs[k][:, j * 512:(j + 1) * 512])
            emit_bn_stats.st = [None] * CK

            def stats_finish(b):
                """bn_aggr + group reduce -> per-channel sc/nb coeffs."""
                mvs = []
                rhs2s = []
                for k in range(CK):
                    mv = small.tile([128, 2], F32, tag=f"mv{k}")
                    nc.vector.bn_aggr(out=mv, in_=emit_bn_stats.st[k])
                    mvs.append(mv)
                    r2 = small.tile([128, 2], F32, tag=f"r2{k}")
                    nc.gpsimd.tensor_copy(out=r2[:, 0:1], in_=mv[:, 0:1])
                    nc.vector.scalar_tensor_tensor(
                        out=r2[:, 1:2], in0=mv[:, 0:1],
                        scalar=mv[:, 0:1], in1=mv[:, 1:2],
                        op0=OP.mult, op1=OP.add)
                    rhs2s.append(r2)
                pg_t = ps_w.tile([128, 512], F32, tag="vp")
                pg = pg_t[0:G, 0:2]
                for k in range(CK):
                    nc.tensor.matmul(pg, indf[k], rhs2s[k],
                                     start=(k == 0), stop=(k == CK - 1))
                sg = small.tile([G, 2], F32, tag="sg")
                nc.vector.tensor_copy(out=sg, in_=pg)
                t2 = small.tile([G, 1], F32, tag="t2")
                nc.vector.tensor_mul(out=t2, in0=sg[:, 0:1], in1=sg[:, 0:1])
                vs = small.tile([G, 1], F32, tag="vs")
                nc.vector.tensor_sub(out=vs, in0=sg[:, 1:2], in1=t2)
                lnv = small.tile([G, 1], F32, tag="lnv")
                nc.scalar.activation(out=lnv, in_=vs, func=AF.Ln,
                                     bias=eps_t[0:G, :], scale=1.0)
                rstd = small.tile([G, 1], F32, tag="rstd")
                nc.scalar.activation(out=rstd, in_=lnv, func=AF.Exp, scale=-0.5)
                bcr = small.tile([G, 2], F32, tag="bcr")
                nc.gpsimd.tensor_copy(out=bcr[:, 0:1], in_=sg[:, 0:1])
                nc.gpsimd.tensor_copy(out=bcr[:, 1:2], in_=rstd)
                scs = []
                nbs = []
                for k in range(CK):
                    pbc_t = ps_w.tile([128, 512], F32, tag="vp")
                    pbc = pbc_t[:, 0:2]
                    nc.tensor.matmul(pbc, indb[:, k * 128:(k + 1) * 128], bcr,
                                     start=True, stop=True)
                    sc = small.tile([128, 1], F32, tag=f"sc{k}")
                    nc.vector.tensor_mul(out=sc, in0=pbc[:, 1:2], in1=gam[k])
                    t4 = small.tile([128, 1], F32, tag=f"t4{k}")
                    nc.vector.tensor_scalar_mul(out=t4, in0=pbc[:, 0:1],
                                                scalar1=sc)
                    nb = small.tile([128, 1], F32, tag=f"nb{k}")
                    nc.vector.tensor_sub(out=nb, in0=bet[k], in1=t4)
                    scs.append(sc)
                    nbs.append(nb)
                return scs, nbs

            def alloc_h(b):
                hs = [hpool.tile([128, N], BF, tag=f"h{k}", name=f"h{k}")
                      for k in range(CK)]
                h8 = [hpool.tile([128, 2, N], F8, tag=f"h8{p}", name=f"h8{p}")
                      for p in range(NP)]
                hps = [small.tile([128, 2], F32, tag=f"hp{k}", name=f"hp{k}")
                       for k in range(CK)]
                return hs, h8, hps

            def norm_op(b, hctx, scs, nbs, k, half):
                """One [128,2048] normalize op on ACT (+accum partial) and
                the matching fp8 normalize on Pool."""
                hs, h8, hps = hctx
                sl = slice(half * 2048, (half + 1) * 2048)
                xs = norm_op.xs
                nc.scalar.activation(
                    out=hs[k][:, sl], in_=xs[k][:, sl], func=AF.Identity,
                    bias=nbs[k], scale=scs[k],
                    accum_out=hps[k][:, half:half + 1])
                nc.gpsimd.tensor_scalar(
                    out=h8[k // 2][:, k % 2, sl], in0=xs[k][:, sl],
                    scalar1=scs[k], scalar2=nbs[k], op0=OP.mult, op1=OP.add)
            norm_op.xs = None

            def hsum_finish(hctx):
                hs, h8, hps = hctx
                hsums = []
                for k in range(CK):
                    t = small.tile([128, 1], BF, tag=f"hsum{k}")
                    nc.vector.tensor_add(out=t, in0=hps[k][:, 0:1],
                                         in1=hps[k][:, 1:2])
                    hsums.append(t)
                return hsums

            def spatial_chunk(b, hctx, Gs, s):
                hs, h8, hps = hctx
                # 4 transposes of h[:, s*128:(s+1)*128] into one bf16 psum tile
                pht = ps_tr.tile([128, 512], BF, tag="tr")
                for k in range(CK):
                    nc.tensor.transpose(
                        pht[:, k * 128:(k + 1) * 128],
                        hs[k][:, s * 128:(s + 1) * 128], ident)
                hT = mid.tile([128, 512], BF, tag="hT")
                # alternate evac engine so neither DVE nor ACT becomes the
                # spatial-loop straggler (DVE also runs bn_stats here)
                if s % 2 == 0:
                    nc.vector.tensor_copy(out=hT, in_=pht)
                else:
                    nc.scalar.copy(out=hT, in_=pht)
                # G[ck] += hT[:, ck].T @ hT
                for k in range(CK):
                    nc.tensor.matmul(Gs[k], hT[:, k * 128:(k + 1) * 128], hT,
                                     start=(s == 0), stop=(s == SP - 1))

            def vproj_t(b, hctx, t):
                hs, h8, hps = hctx
                vts = []
                for oc in range(CK):
                    pv = ps_w.tile([128, 512], F32, tag="vp")
                    for p in range(NP):
                        nc.tensor.matmul(
                            pv, wv8[p][:, :, oc * 128:(oc + 1) * 128],
                            h8[p][:, :, t * 512:(t + 1) * 512],
                            start=(p == 0), stop=(p == NP - 1), perf_mode=DR)
                    vt = vpool.tile([128, 512], BF, tag=f"v{oc}_{t}")
                    nc.scalar.mul(out=vt, in_=pv, mul=IWSC)
                    vts.append(vt)
                return vts

            def gram_finish(b, Gs, hsums):
                # evac G (bf16, symmetric)
                G_sb = []
                for k in range(CK):
                    t = gpool.tile([128, 512], BF, tag=f"G{k}")
                    nc.scalar.copy(out=t, in_=Gs[k])
                    G_sb.append(t)
                # qks row = hsum^T @ wqkT  -> [1, 2C]
                pq_t = ps_w.tile([128, 512], F32, tag="av")
                qks_sb = gpool.tile([1, 2 * C], BF, tag="qks")
                for half in range(2):
                    pq = pq_t[0:1, :]
                    for k in range(CK):
                        nc.tensor.matmul(
                            pq, hsums[k],
                            wqk[k][:, half * 512:(half + 1) * 512],
                            start=(k == 0), stop=(k == CK - 1))
                    nc.vector.tensor_copy(
                        out=qks_sb[:, half * 512:(half + 1) * 512], in_=pq)
                # ks2 = ks + N*bk
                ks2 = gpool.tile([1, C], BF, tag="ks2")
                nc.vector.scalar_tensor_tensor(
                    out=ks2, in0=bqkr[:, C:2 * C], scalar=float(N),
                    in1=qks_sb[:, C:2 * C], op0=OP.mult, op1=OP.add)
                # T = G @ Wk^T  (G symmetric: lhsT = G_sb[a][:, m-chunk])
                T_sb = []
                for m in range(CK):
                    pT = ps_w.tile([128, 512], F32, tag="op")
                    for a in range(CK):
                        nc.tensor.matmul(
                            pT, G_sb[a][:, m * 128:(m + 1) * 128],
                            wqk[a][:, C:2 * C],
                            start=(a == 0), stop=(a == CK - 1))
                    t = gpool.tile([128, 512], BF, tag=f"T{m}")
                    nc.scalar.copy(out=t, in_=pT)
                    T_sb.append(t)
                # scores: per chunk ck, heads 2ck (even rows) / 2ck+1 (odd)
                # shares the "av" bank: qks (before) and cv (after) don't
                # overlap its lifetime
                SC = ps_w.tile([128, 256], F32, tag="av")
                for ck in range(CK):
                    for par in range(2):
                        hh = 2 * ck + par
                        hsl = slice(hh * 64, (hh + 1) * 64)
                        out_ap = SC[par * 64:(par + 1) * 64,
                                    ck * 64:(ck + 1) * 64]
                        tp = (0, par * 64)
                        nc.tensor.matmul(
                            out_ap, bqkr[:, hsl], ks2[:, hsl],
                            start=True, stop=False, tile_position=tp,
                            skip_group_check=True)
                        nc.tensor.matmul(
                            out_ap, qks_sb[:, hsl], bqkr[:, C + hh * 64:
                                                         C + (hh + 1) * 64],
                            start=False, stop=False, tile_position=tp,
                            skip_group_check=True)
                        for a in range(CK):
                            nc.tensor.matmul(
                                out_ap, wqk[a][:, hsl], T_sb[a][:, hsl],
                                start=False, stop=(a == CK - 1),
                                tile_position=tp, skip_group_check=True)
                return SC

            def softmax(b, SC):
                p_f = mid.tile([128, 256], F32, tag="pf", bufs=1)
                rs = mid.tile([128, CK], F32, tag="rs", bufs=1)
                for ck in range(CK):
                    for par in range(2):
                        rsl = slice(par * 64, (par + 1) * 64)
                        nc.scalar.activation(
                            out=p_f[rsl, ck * 64:(ck + 1) * 64],
                            in_=SC[rsl, ck * 64:(ck + 1) * 64],
                            func=AF.Exp, scale=scale,
                            accum_out=rs[rsl, ck:ck + 1])
                rv = mid.tile([128, CK], F32, tag="rv", bufs=1)
                nc.vector.reciprocal(out=rv, in_=rs)
                for ck in range(CK):
                    for par in range(2):
                        rsl = slice(par * 64, (par + 1) * 64)
                        nc.vector.tensor_scalar_mul(
                            out=att_bf[ck][rsl, par * 64:(par + 1) * 64],
                            in0=p_f[rsl, ck * 64:(ck + 1) * 64],
                            scalar1=rv[rsl, ck:ck + 1])

            def att_transpose(b):
                patt = ps_tr.tile([128, 512], BF, tag="tr")
                for ck in range(CK):
                    nc.tensor.transpose(
                        patt[:, ck * 128:(ck + 1) * 128], att_bf[ck], ident)
                attT = mid.tile([128, 512], BF, tag="attT", bufs=1)
                nc.vector.tensor_copy(out=attT, in_=patt)
                # cv = attT.T(!) applied to b_v: one matmul per chunk
                pcv_t = ps_w.tile([128, 512], F32, tag="av")
                for ck in range(CK):
                    nc.tensor.matmul(
                        pcv_t[:, ck:ck + 1],
                        attT[:, ck * 128:(ck + 1) * 128], bv_sb[ck],
                        start=True, stop=True, skip_group_check=True)
                cs4 = small.tile([128, CK], F32, tag="cs4")
                nc.vector.tensor_copy(out=cs4, in_=pcv_t[:, 0:CK])
                return attT, cs4

            def att_out_t(b, attT, cs4, vsave, hv8, t, norm_cb=None):
                # att @ v for the 4 chunks of this t-block
                for ck in range(CK):
                    pav = ps_w.tile([128, 512], F32, tag="av")
                    nc.tensor.matmul(
                        pav, attT[:, ck * 128:(ck + 1) * 128],
                        vsave[t][ck], start=True, stop=True)
                    nc.vector.tensor_scalar_add(
                        out=hv8[ck // 2][:, ck % 2, t * 512:(t + 1) * 512],
                        in0=pav, scalar1=cs4[:, ck:ck + 1])
                if norm_cb is not None:
                    norm_cb()  # one normalize slab of the next batch on ACT
                for oc in range(CK):
                    po = ps_w.tile([128, 512], F32, tag="op")
                    for p in range(NP):
                        nc.tensor.matmul(
                            po, wo8[p][:, :, oc * 128:(oc + 1) * 128],
                            hv8[p][:, :, t * 512:(t + 1) * 512],
                            start=(p == 0), stop=(p == NP - 1), perf_mode=DR)
                    ot = mid.tile([128, 512], BF, tag="ot", bufs=2)
                    nc.scalar.activation(out=ot, in_=po, func=AF.Identity,
                                         bias=bo_sb[oc], scale=IWSC)
                    xr = mid.tile([128, 512], F32, tag="xr", bufs=2)
                    nc.sync.dma_start(
                        out=xr,
                        in_=xf_d.ap()[b, oc * 128:(oc + 1) * 128,
                                      t * 512:(t + 1) * 512])
                    fin = mid.tile([128, 512], F32, tag="fin", bufs=2)
                    nc.gpsimd.tensor_add(out=fin, in0=xr, in1=ot)
                    dma_eng = nc.gpsimd if b < B - 1 else nc.sync
                    dma_eng.dma_start(
                        out=out_d.ap()[b, oc * 128:(oc + 1) * 128,
                                       t * 512:(t + 1) * 512],
                        in_=fin)

            # ---------------- pipelined emission ----------------
            # batch 0 prologue
            xs = load_x(0)
            norm_op.xs = xs
            for k in range(CK):
                for j in range(SUB):
                    emit_bn_stats(xs, k, j)
            scs, nbs = stats_finish(0)
            hctx = alloc_h(0)
            for k in range(CK):
                for half in range(2):
                    norm_op(0, hctx, scs, nbs, k, half)
            hsums = hsum_finish(hctx)

            prev = None  # (attT, cs4, vsave, hv8) of batch b-1
            for b in range(B):
                Gs = [ps_g.tile([128, 512], F32, tag=f"G{k}", name=f"G{k}")
                      for k in range(CK)]
                vsave = [None] * NT
                hv8 = [hpool.tile([128, 2, N], F8, tag=f"hv8{p}",
                                  name=f"hv8{p}")
                       for p in range(NP)]
                nxt_stats_ops = []
                if b + 1 < B:
                    # spread next batch's x load + bn_stats over this loop
                    nxt_stats_ops = [(k, j) for k in range(CK)
                                     for j in range(SUB)]
                for s in range(SP):
                    if b + 1 < B and s == 0:
                        xs_n = load_x(b + 1)
                    spatial_chunk(b, hctx, Gs, s)
                    if s % 4 == 3:
                        t = s // 4
                        vsave[t] = vproj_t(b, hctx, t)
                    if b + 1 < B and s >= SP - 16:
                        # 2 bn_stats ops per spatial chunk in the tail half
                        for _ in range(2):
                            if nxt_stats_ops:
                                k, j = nxt_stats_ops.pop(0)
                                emit_bn_stats(xs_n, k, j)
                SCp = gram_finish(b, Gs, hsums)
                softmax(b, SCp)
                if b + 1 < B:
                    scs, nbs = stats_finish(b + 1)
                    norm_op.xs = xs_n
                    hctx_n = alloc_h(b + 1)
                    norm_jobs = [(k, half) for k in range(CK)
                                 for half in range(2)]
                else:
                    hctx_n = None
                    norm_jobs = []
                attT, cs4 = att_transpose(b)
                for t in range(NT):
                    if norm_jobs:
                        k, half = norm_jobs.pop(0)
                        cb = (lambda k=k, half=half:
                              norm_op(b + 1, hctx_n, scs, nbs, k, half))
                    else:
                        cb = None
                    att_out_t(b, attT, cs4, vsave, hv8, t, norm_cb=cb)
                if b + 1 < B:
                    hctx = hctx_n
                    hsums = hsum_finish(hctx)

    nc.compile()
    return nc


def make_indicators():
    ch = np.arange(C)
    grp = ch // (C // G)
    indf = np.zeros((C, G), np.float32)
    indf[ch, grp] = 1.0 / (C // G)
    indb = np.zeros((G, C), np.float32)
    indb[grp, ch] = 1.0
    return indf, indb


def prep_weights(w_qkv, b_qkv, w_out, b_out, gamma, beta):
    """Host-side weight layouts. Returns dict of per-core input tensors
    (excluding x)."""
    bf = ml_dtypes.bfloat16
    f8 = ml_dtypes.float8_e4m3
    w_qkv = np.asarray(w_qkv, np.float32)
    wqkT = np.ascontiguousarray(w_qkv[:2 * C].T).astype(bf)

    def pack_dr(wT):
        # wT [C, C] (contraction-major) -> [NP, 128, 2, C] fp8 scaled
        a = (np.asarray(wT, np.float32) * WSC).reshape(NP, 2, 128, C)
        return np.ascontiguousarray(a.transpose(0, 2, 1, 3)).astype(f8)

    wv8 = pack_dr(w_qkv[2 * C:].T)
    wo8 = pack_dr(np.asarray(w_out, np.float32).T)
    b_qkv = np.asarray(b_qkv, np.float32)
    indf, indb = make_indicators()
    return {
        "wqkT": wqkT, "wv8": wv8, "wo8": wo8,
        "bqkr": np.ascontiguousarray(b_qkv[:2 * C].reshape(1, -1)).astype(bf),
        "bv": np.ascontiguousarray(b_qkv[2 * C:].reshape(-1, 1)).astype(bf),
        "bo": np.ascontiguousarray(np.asarray(b_out, np.float32).reshape(-1, 1)),
        "gamma": np.ascontiguousarray(np.asarray(gamma, np.float32).reshape(-1, 1)),
        "beta": np.ascontiguousarray(np.asarray(beta, np.float32).reshape(-1, 1)),
        "indf": indf, "indb": indb,
    }


_PROGRAM = None


def _get_program():
    global _PROGRAM
    if _PROGRAM is None:
        _PROGRAM = build_program()
    return _PROGRAM


def kernel(x, gamma, beta, w_qkv, b_qkv, w_out, b_out):
    x = np.asarray(x)
    B, C_, H, W = x.shape
    N = H * W
    assert C_ == C and B == 16 and N == 4096
    nc = _get_program()

    bf = ml_dtypes.bfloat16
    wd = prep_weights(w_qkv, b_qkv, w_out, b_out, gamma, beta)
    xr = np.ascontiguousarray(x.reshape(B, C, N).astype(np.float32))
    xb = xr.astype(bf)

    bpc = B // N_CORES
    in_maps = []
    for c in range(N_CORES):
        m = {"xbf": xb[c * bpc:(c + 1) * bpc],
             "xf": xr[c * bpc:(c + 1) * bpc]}
        m.update(wd)
        in_maps.append(m)
    res = run_bass_kernel_spmd(nc, in_maps, core_ids=list(range(N_CORES)))
    out = np.concatenate([res.results[c]["out"] for c in range(N_CORES)],
                         axis=0)
    return out.reshape(B, C_, H, W).astype(np.float32)
